# revision 1
# baseline (speedup 1.0000x reference)
"""CrossViewAttention Trainium2 kernel.

Sharding: 8 NeuronCores = 2 batches x 4 attention heads. Each core runs the
dominant attention compute (scores = qh.kh per camera, exp, P@[V|1] with the
softmax denominator fused as a 33rd output row) over all 6*1680 keys for its
(batch, head). Light geometry / BN-conv / LN projections and the output
proj+MLP run on host numpy (<3% of FLOPs).

Self-contained: hardcodes all shapes; no sibling imports.
"""
import sys, os
sys.path.insert(0, "/opt/trn_rl_repo")

import numpy as np
import ml_dtypes
from scipy.special import erf

B, N, C_FEAT, FH, FW = 2, 6, 128, 28, 60
D, HEADS, DHEAD = 128, 4, 32
BH, BW = 32, 32
EPS = 1e-5
K = FH * FW            # 1680 keys per camera
Q = BH * BW            # 1024 queries
NK = N * K             # 10080
PCH = 120              # pixel chunk (1680 = 14*120 -> camera-aligned chunks)
NCH = NK // PCH        # 84 chunks
CH_PER_CAM = K // PCH  # 14

_bf16 = ml_dtypes.bfloat16
_CACHE = {}


def _build_nc():
    import concourse.tile as tile
    from concourse import bacc, mybir

    nc = bacc.Bacc("TRN2", target_bir_lowering=False, debug=False, num_devices=1)
    dt = mybir.dt
    kh = nc.dram_tensor("kh", [DHEAD, NK], dt.bfloat16, kind="ExternalInput").ap()
    qh = nc.dram_tensor("qh", [DHEAD, N * Q], dt.bfloat16, kind="ExternalInput").ap()
    vo = nc.dram_tensor("vo", [NCH * PCH, DHEAD + 1], dt.bfloat16, kind="ExternalInput").ap()
    av = nc.dram_tensor("av", [DHEAD + 1, Q], dt.float32, kind="ExternalOutput").ap()

    SCALE = 1.0 / np.sqrt(DHEAD)
    with tile.TileContext(nc) as tc:
        with (
            tc.tile_pool(name="kq", bufs=1) as kq_pool,
            tc.tile_pool(name="p", bufs=3) as p_pool,
            tc.tile_pool(name="ps", bufs=2, space="PSUM") as ps_pool,
            tc.tile_pool(name="avp", bufs=1, space="PSUM") as av_pool,
            tc.tile_pool(name="outs", bufs=1) as out_pool,
        ):
            kh_sb = kq_pool.tile([DHEAD, NK], dt.bfloat16, tag="kh")
            nc.sync.dma_start(kh_sb[:], kh[:])
            qh_sb = kq_pool.tile([DHEAD, N * Q], dt.bfloat16, tag="qh")
            nc.sync.dma_start(qh_sb[:], qh[:])
            vo_sb = kq_pool.tile([PCH, NCH * (DHEAD + 1)], dt.bfloat16, tag="vo")
            # vo dram is [NCH*PCH, 33]; load chunk c into partitions 0..119,
            # free cols 33c..33c+33
            vo_r = vo.rearrange("(c p) m -> c p m", p=PCH)
            for c in range(NCH):
                nc.sync.dma_start(vo_sb[:, c * (DHEAD + 1):(c + 1) * (DHEAD + 1)], vo_r[c])

            av_ps = av_pool.tile([DHEAD + 1, Q], dt.float32)
            for c in range(NCH):
                cam = c // CH_PER_CAM
                s_ps = ps_pool.tile([PCH, Q], dt.float32, tag="scores")
                for half in range(2):
                    nc.tensor.matmul(
                        s_ps[:, half * 512:(half + 1) * 512],
                        kh_sb[:, c * PCH:(c + 1) * PCH],
                        qh_sb[:, cam * Q + half * 512: cam * Q + (half + 1) * 512],
                        start=True, stop=True,
                    )
                p_sb = p_pool.tile([PCH, Q], dt.bfloat16, tag="p")
                nc.scalar.activation(
                    p_sb[:], s_ps[:], mybir.ActivationFunctionType.Exp, scale=SCALE
                )
                for half in range(2):
                    nc.tensor.matmul(
                        av_ps[:, half * 512:(half + 1) * 512],
                        vo_sb[:, c * (DHEAD + 1):(c + 1) * (DHEAD + 1)],
                        p_sb[:, half * 512:(half + 1) * 512],
                        start=(c == 0), stop=(c == NCH - 1),
                    )
            av_sb = out_pool.tile([DHEAD + 1, Q], dt.float32)
            nc.vector.tensor_copy(av_sb[:], av_ps[:])
            nc.sync.dma_start(av[:], av_sb[:])

    nc.compile()
    return nc


def _ln(t, g, b):
    mu = t.mean(-1, keepdims=True)
    var = ((t - mu) ** 2).mean(-1, keepdims=True)
    return (t - mu) / np.sqrt(var + EPS) * g + b


def _bn_relu_conv(t, g, b, m, v, w):
    # t: (x, C, K)
    s = g / np.sqrt(v + EPS)
    th = t * s[:, None] + (b - m * s)[:, None]
    return np.einsum("oc,xck->xok", w, np.maximum(th, 0.0), optimize=True)


def kernel(**inputs):
    inp = {k: np.asarray(v, dtype=np.float32) for k, v in inputs.items()}
    x = inp["x"]; feature = inp["feature"]; I_inv = inp["I_inv"]; E_inv = inp["E_inv"]
    bev_grid = inp["bev_grid"]; image_plane = inp["image_plane"]

    # --- host: geometry embeddings ---
    c = E_inv[..., -1]                                        # (b,n,4)
    c_embed = np.einsum("oc,bnc->bno", inp["cam_w"], c)       # (b,n,128)
    pixp = image_plane.reshape(3, K)
    cam = np.einsum("bnij,jk->bnik", I_inv, pixp)
    cam = np.concatenate([cam, np.ones((B, N, 1, K), np.float32)], 2)
    d = np.einsum("bnij,bnjk->bnik", E_inv, cam)
    d_embed = np.einsum("oc,bnck->bnok", inp["img_w"], d)     # (b,n,128,K)
    img_embed = d_embed - c_embed[..., None]
    img_embed = img_embed / (np.linalg.norm(img_embed, axis=2, keepdims=True) + 1e-7)
    w_embed = np.einsum("oc,chw->ohw", inp["bev_w"], bev_grid[:2]) + inp["bev_b"][:, None, None]
    bev_embed = w_embed.reshape(1, 1, D, Q) - c_embed[..., None]
    bev_embed = bev_embed / (np.linalg.norm(bev_embed, axis=2, keepdims=True) + 1e-7)

    feat = feature.reshape(B * N, C_FEAT, K)
    key_flat = img_embed + _bn_relu_conv(
        feat, inp["fp_bn_g"], inp["fp_bn_b"], inp["fp_bn_m"], inp["fp_bn_v"], inp["fp_w"]
    ).reshape(B, N, D, K)
    val_flat = _bn_relu_conv(
        feat, inp["fl_bn_g"], inp["fl_bn_b"], inp["fl_bn_m"], inp["fl_bn_v"], inp["fl_w"]
    ).reshape(B, N, D, K)
    query = bev_embed + x.reshape(B, 1, D, Q)

    # --- host: LN + qkv projections ---
    q = query.reshape(B, N, D, Q).transpose(0, 1, 3, 2)       # (b,n,Q,128)
    k = key_flat.transpose(0, 1, 3, 2)                        # (b,n,K,128)
    v = val_flat.transpose(0, 1, 3, 2).reshape(B, NK, D)
    q = _ln(q, inp["q_ln_g"], inp["q_ln_b"]) @ inp["q_w"] + inp["q_b"]
    k = _ln(k, inp["k_ln_g"], inp["k_ln_b"]) @ inp["k_w"] + inp["k_b"]
    v = _ln(v, inp["v_ln_g"], inp["v_ln_b"]) @ inp["v_w"] + inp["v_b"]
    qh = q.reshape(B, N, Q, HEADS, DHEAD)
    kh = k.reshape(B, N, K, HEADS, DHEAD).reshape(B, NK, HEADS, DHEAD)
    vh = v.reshape(B, NK, HEADS, DHEAD)

    # --- device: per (b,h) attention with fused denominator ---
    in_maps = []
    for core in range(8):
        b, h = core // HEADS, core % HEADS
        kh_d = np.ascontiguousarray(kh[b, :, h, :].T).astype(_bf16)      # [32, NK]
        qh_d = np.ascontiguousarray(
            qh[b, :, :, h, :].transpose(2, 0, 1).reshape(DHEAD, N * Q)
        ).astype(_bf16)                                                   # [32, N*Q]
        vo_d = np.concatenate(
            [vh[b, :, h, :], np.ones((NK, 1), np.float32)], 1
        ).astype(_bf16)                                                   # [NK, 33]
        in_maps.append({"kh": kh_d, "qh": qh_d, "vo": vo_d})

    if os.environ.get("KERNEL_EMULATE"):
        avs = []
        for core in range(8):
            m = in_maps[core]
            s = m["kh"].astype(np.float32).T @ m["qh"].astype(np.float32)[:, :0]
            khf = m["kh"].astype(np.float32)          # [32, NK]
            qhf = m["qh"].astype(np.float32)          # [32, N*Q]
            vof = m["vo"].astype(np.float32)          # [NK, 33]
            av = np.zeros((DHEAD + 1, Q), np.float32)
            for ci in range(NCH):
                camn = ci // CH_PER_CAM
                sc = khf[:, ci * PCH:(ci + 1) * PCH].T @ qhf[:, camn * Q:(camn + 1) * Q]
                p = np.exp(sc / np.sqrt(DHEAD)).astype(_bf16).astype(np.float32)
                av += vof[ci * PCH:(ci + 1) * PCH].T @ p
            avs.append(av)
    else:
        import time
        from concourse.bass_utils import run_bass_kernel_spmd
        if "nc" not in _CACHE:
            _CACHE["nc"] = _build_nc()
        t0 = time.time()
        res = run_bass_kernel_spmd(_CACHE["nc"], in_maps, core_ids=list(range(8)))
        _CACHE["device_wall_s"] = time.time() - t0
        avs = [res.results[i]["av"] for i in range(8)]

    # --- host: combine heads, proj, MLP ---
    a = np.zeros((B, Q, HEADS, DHEAD), np.float32)
    for core in range(8):
        b, h = core // HEADS, core % HEADS
        av = avs[core]
        a[b, :, h, :] = (av[:DHEAD] / av[DHEAD:DHEAD + 1]).T
    a = a.reshape(B, Q, HEADS * DHEAD)
    z = a @ inp["proj_w"] + inp["proj_b"]
    z = z + x.reshape(B, D, Q).transpose(0, 2, 1)
    z = _ln(z, inp["pre_g"], inp["pre_b"])
    h1 = z @ inp["mlp_w1"] + inp["mlp_b1"]
    h1 = 0.5 * h1 * (1.0 + erf(h1 / np.sqrt(2.0)))
    z = z + h1 @ inp["mlp_w2"] + inp["mlp_b2"]
    z = _ln(z, inp["post_g"], inp["post_b"])
    return z.transpose(0, 2, 1).reshape(B, D, BH, BW).astype(np.float32)



# revision 3
# speedup vs baseline: 1.9048x; 1.9048x over previous
"""CrossViewAttention Trainium2 kernel.

Sharding: 8 NeuronCores = 2 batches x 4 attention heads. Each core runs the
dominant attention compute (scores = qh.kh per camera, exp, P@[V|1] with the
softmax denominator fused as a 33rd output row) over all 6*1680 keys for its
(batch, head). Light geometry / BN-conv / LN projections and the output
proj+MLP run on host numpy (<3% of FLOPs), kept channels-first to avoid
large transposes. The PJRT dispatch callable is built once and cached so
warm calls skip JAX retrace/XLA rebuild.

Self-contained: hardcodes all shapes; no sibling imports.
"""
import sys, os
sys.path.insert(0, "/opt/trn_rl_repo")

import numpy as np
import ml_dtypes
from scipy.special import erf

B, N, C_FEAT, FH, FW = 2, 6, 128, 28, 60
D, HEADS, DHEAD = 128, 4, 32
BH, BW = 32, 32
EPS = 1e-5
K = FH * FW            # 1680 keys per camera
Q = BH * BW            # 1024 queries
NK = N * K             # 10080
PCH = 120              # pixel chunk (1680 = 14*120 -> camera-aligned chunks)
NCH = NK // PCH        # 84 chunks
CH_PER_CAM = K // PCH  # 14

_bf16 = ml_dtypes.bfloat16
_CACHE = {}


def _build_nc():
    import concourse.tile as tile
    from concourse import bacc, mybir

    nc = bacc.Bacc("TRN2", target_bir_lowering=False, debug=False, num_devices=1)
    dt = mybir.dt
    kh = nc.dram_tensor("kh", [DHEAD, NK], dt.bfloat16, kind="ExternalInput").ap()
    qh = nc.dram_tensor("qh", [DHEAD, N * Q], dt.bfloat16, kind="ExternalInput").ap()
    vo = nc.dram_tensor("vo", [NCH * PCH, DHEAD + 1], dt.bfloat16, kind="ExternalInput").ap()
    av = nc.dram_tensor("av", [DHEAD + 1, Q], dt.float32, kind="ExternalOutput").ap()

    SCALE = 1.0 / np.sqrt(DHEAD)
    with tile.TileContext(nc) as tc:
        with (
            tc.tile_pool(name="kq", bufs=1) as kq_pool,
            tc.tile_pool(name="p", bufs=3) as p_pool,
            tc.tile_pool(name="ps", bufs=2, space="PSUM") as ps_pool,
            tc.tile_pool(name="avp", bufs=1, space="PSUM") as av_pool,
            tc.tile_pool(name="outs", bufs=1) as out_pool,
        ):
            kh_sb = kq_pool.tile([DHEAD, NK], dt.bfloat16, tag="kh")
            nc.sync.dma_start(kh_sb[:], kh[:])
            qh_sb = kq_pool.tile([DHEAD, N * Q], dt.bfloat16, tag="qh")
            nc.sync.dma_start(qh_sb[:], qh[:])
            vo_sb = kq_pool.tile([PCH, NCH * (DHEAD + 1)], dt.bfloat16, tag="vo")
            # vo dram is [NCH*PCH, 33]; load chunk c into partitions 0..119,
            # free cols 33c..33c+33
            vo_r = vo.rearrange("(c p) m -> c p m", p=PCH)
            for c in range(NCH):
                nc.sync.dma_start(vo_sb[:, c * (DHEAD + 1):(c + 1) * (DHEAD + 1)], vo_r[c])

            av_ps = av_pool.tile([DHEAD + 1, Q], dt.float32)
            for c in range(NCH):
                cam = c // CH_PER_CAM
                s_ps = ps_pool.tile([PCH, Q], dt.float32, tag="scores")
                for half in range(2):
                    nc.tensor.matmul(
                        s_ps[:, half * 512:(half + 1) * 512],
                        kh_sb[:, c * PCH:(c + 1) * PCH],
                        qh_sb[:, cam * Q + half * 512: cam * Q + (half + 1) * 512],
                        start=True, stop=True,
                    )
                p_sb = p_pool.tile([PCH, Q], dt.bfloat16, tag="p")
                nc.scalar.activation(
                    p_sb[:], s_ps[:], mybir.ActivationFunctionType.Exp, scale=SCALE
                )
                for half in range(2):
                    nc.tensor.matmul(
                        av_ps[:, half * 512:(half + 1) * 512],
                        vo_sb[:, c * (DHEAD + 1):(c + 1) * (DHEAD + 1)],
                        p_sb[:, half * 512:(half + 1) * 512],
                        start=(c == 0), stop=(c == NCH - 1),
                    )
            av_sb = out_pool.tile([DHEAD + 1, Q], dt.float32)
            nc.vector.tensor_copy(av_sb[:], av_ps[:])
            nc.sync.dma_start(av[:], av_sb[:])

    nc.compile()
    return nc


def _build_dispatch(nc):
    """Build the sharded PJRT callable ONCE (what run_bass_kernel_spmd
    re-creates per call under axon) and return a fast-path runner."""
    import jax
    from jax.sharding import Mesh, PartitionSpec
    from jax.experimental.shard_map import shard_map
    from concourse.bass2jax import (
        _bass_exec_p, install_neuronx_cc_hook, partition_id_tensor,
    )
    from concourse import mybir

    install_neuronx_cc_hook()
    partition_name = nc.partition_id_tensor.name if nc.partition_id_tensor else None
    in_names, out_names, out_avals, zero_shapes = [], [], [], []
    for alloc in nc.m.functions[0].allocations:
        if not isinstance(alloc, mybir.MemoryLocationSet):
            continue
        name = alloc.memorylocations[0].name
        if alloc.kind == "ExternalInput":
            if name != partition_name:
                in_names.append(name)
        elif alloc.kind == "ExternalOutput":
            shape = tuple(alloc.tensor_shape)
            dtype = mybir.dt.np(alloc.dtype)
            out_names.append(name)
            out_avals.append(jax.core.ShapedArray(shape, dtype))
            zero_shapes.append((shape, dtype))
    n_params = len(in_names)
    n_outs = len(out_avals)
    in_names_all = in_names + out_names
    if partition_name is not None:
        in_names_all.append(partition_name)
    donate = tuple(range(n_params, n_params + n_outs))

    def _body(*args):
        operands = list(args)
        if partition_name is not None:
            operands.append(partition_id_tensor())
        outs = _bass_exec_p.bind(
            *operands,
            out_avals=tuple(out_avals),
            in_names=tuple(in_names_all),
            out_names=tuple(out_names),
            lowering_input_output_aliases=(),
            sim_require_finite=True,
            sim_require_nnan=True,
            nc=nc,
        )
        return tuple(outs)

    devices = jax.devices()[:8]
    mesh = Mesh(np.asarray(devices), ("core",))
    in_specs = (PartitionSpec("core"),) * (n_params + n_outs)
    out_specs = (PartitionSpec("core"),) * len(out_names)
    sharded = jax.jit(
        shard_map(_body, mesh=mesh, in_specs=in_specs, out_specs=out_specs,
                  check_rep=False),
        donate_argnums=donate,
        keep_unused=True,
    )

    def run(in_maps):
        concat_in = [
            np.concatenate([np.asarray(m[name]) for m in in_maps], axis=0)
            for name in in_names
        ]
        concat_zeros = [
            np.zeros((8 * s[0], *s[1:]), dt) for s, dt in zero_shapes
        ]
        out_arrs = sharded(*concat_in, *concat_zeros)
        return [
            {
                name: np.asarray(out_arrs[i]).reshape(8, *out_avals[i].shape)[c]
                for i, name in enumerate(out_names)
            }
            for c in range(8)
        ]

    return run


def _ln_cf(t, g, b):
    # LayerNorm over channel axis 1 of (X, 128, M), in-place friendly
    mu = t.mean(1, keepdims=True)
    t = t - mu
    var = np.einsum("xcm,xcm->xm", t, t) / t.shape[1]
    t *= (1.0 / np.sqrt(var + EPS))[:, None, :]
    if g is not None:
        t *= g[:, None]
    if b is not None:
        t += b[:, None]
    return t


def kernel(**inputs):
    inp = {k: np.asarray(v, dtype=np.float32) for k, v in inputs.items()}
    x = inp["x"]; feature = inp["feature"]; I_inv = inp["I_inv"]; E_inv = inp["E_inv"]
    bev_grid = inp["bev_grid"]; image_plane = inp["image_plane"]

    # --- host: geometry embeddings (channels-first throughout) ---
    c = E_inv[..., -1]                                        # (b,n,4)
    c_embed = np.einsum("oc,bnc->bno", inp["cam_w"], c)       # (b,n,128)
    pixp = image_plane.reshape(3, K)
    cam = np.einsum("bnij,jk->bnik", I_inv, pixp)
    cam = np.concatenate([cam, np.ones((B, N, 1, K), np.float32)], 2)
    d = np.einsum("bnij,bnjk->bnik", E_inv, cam)              # (b,n,4,K)
    d_embed = np.einsum("oc,bnck->bnok", inp["img_w"], d)     # (b,n,128,K)
    img_embed = d_embed - c_embed[..., None]
    img_embed /= np.sqrt(np.einsum("bnck,bnck->bnk", img_embed, img_embed))[:, :, None, :] + 1e-7
    w_embed = (inp["bev_w"] @ bev_grid[:2].reshape(2, Q)) + inp["bev_b"][:, None]
    bev_embed = w_embed.reshape(1, 1, D, Q) - c_embed[..., None]
    bev_embed /= np.sqrt(np.einsum("bnck,bnck->bnk", bev_embed, bev_embed))[:, :, None, :] + 1e-7

    # --- host: BN->ReLU->1x1conv, channels-first, batched BLAS ---
    feat = feature.reshape(B * N, C_FEAT, K)
    def bn_relu_conv(p):
        s = inp[p + "_bn_g"] / np.sqrt(inp[p + "_bn_v"] + EPS)
        t = inp[p + "_bn_b"] - inp[p + "_bn_m"] * s
        th = feat * s[:, None] + t[:, None]
        np.maximum(th, 0.0, out=th)
        return np.matmul(inp[p + "_w"][None], th)             # (12,128,K)
    key_flat = img_embed.reshape(B * N, D, K) + bn_relu_conv("fp")
    val_flat = bn_relu_conv("fl")
    query = (bev_embed + x.reshape(B, 1, D, Q)).reshape(B * N, D, Q)

    # --- host: LN (over channels) + qkv projections, channels-first ---
    qf = _ln_cf(query, inp["q_ln_g"], inp["q_ln_b"])
    kf = _ln_cf(key_flat, inp["k_ln_g"], inp["k_ln_b"])
    vf = _ln_cf(val_flat, inp["v_ln_g"], inp["v_ln_b"])
    # out (12, md, M) = w.T @ ln
    qt = np.matmul(inp["q_w"].T[None], qf) + inp["q_b"][:, None]
    kt = np.matmul(inp["k_w"].T[None], kf) + inp["k_b"][:, None]
    vt = np.matmul(inp["v_w"].T[None], vf) + inp["v_b"][:, None]
    qt = qt.reshape(B, N, D, Q).astype(_bf16)
    kt = kt.reshape(B, N, D, K).astype(_bf16)
    vt = vt.reshape(B, N, D, K).astype(_bf16)

    # --- device in_maps: per (b,h) head slices ---
    in_maps = []
    ones_col = np.ones((NK, 1), _bf16)
    for core in range(8):
        b, h = core // HEADS, core % HEADS
        sl = slice(h * DHEAD, (h + 1) * DHEAD)
        kh_d = np.ascontiguousarray(
            kt[b, :, sl, :].transpose(1, 0, 2).reshape(DHEAD, NK))
        qh_d = np.ascontiguousarray(
            qt[b, :, sl, :].transpose(1, 0, 2).reshape(DHEAD, N * Q))
        vo_d = np.empty((NK, DHEAD + 1), _bf16)
        vo_d[:, :DHEAD] = vt[b, :, sl, :].transpose(0, 2, 1).reshape(NK, DHEAD)
        vo_d[:, DHEAD:] = ones_col
        in_maps.append({"kh": kh_d, "qh": qh_d, "vo": vo_d})

    if os.environ.get("KERNEL_EMULATE"):
        avs = []
        for core in range(8):
            m = in_maps[core]
            khf = m["kh"].astype(np.float32)          # [32, NK]
            qhf = m["qh"].astype(np.float32)          # [32, N*Q]
            vof = m["vo"].astype(np.float32)          # [NK, 33]
            av = np.zeros((DHEAD + 1, Q), np.float32)
            for camn in range(N):
                sc = khf[:, camn * K:(camn + 1) * K].T @ qhf[:, camn * Q:(camn + 1) * Q]
                p = np.exp(sc / np.sqrt(DHEAD))
                av += vof[camn * K:(camn + 1) * K].T @ p
            avs.append(av)
    else:
        import time
        if "run" not in _CACHE:
            _CACHE["nc"] = _build_nc()
            _CACHE["run"] = _build_dispatch(_CACHE["nc"])
        t0 = time.time()
        res = _CACHE["run"](in_maps)
        _CACHE["device_wall_s"] = time.time() - t0
        avs = [res[i]["av"] for i in range(8)]

    # --- host: combine heads, proj, MLP ---
    a = np.empty((B, Q, HEADS, DHEAD), np.float32)
    for core in range(8):
        b, h = core // HEADS, core % HEADS
        av = avs[core]
        a[b, :, h, :] = (av[:DHEAD] / av[DHEAD:DHEAD + 1]).T
    a = a.reshape(B, Q, HEADS * DHEAD)
    z = a @ inp["proj_w"] + inp["proj_b"]
    z += x.reshape(B, D, Q).transpose(0, 2, 1)
    mu = z.mean(-1, keepdims=True)
    z -= mu
    var = np.einsum("bqc,bqc->bq", z, z) / D
    z *= (1.0 / np.sqrt(var + EPS))[..., None]
    z = z * inp["pre_g"] + inp["pre_b"]
    h1 = z @ inp["mlp_w1"] + inp["mlp_b1"]
    h1 = 0.5 * h1 * (1.0 + erf(h1 * np.float32(1.0 / np.sqrt(2.0))))
    z = z + h1 @ inp["mlp_w2"] + inp["mlp_b2"]
    mu = z.mean(-1, keepdims=True)
    z -= mu
    var = np.einsum("bqc,bqc->bq", z, z) / D
    z *= (1.0 / np.sqrt(var + EPS))[..., None]
    z = z * inp["post_g"] + inp["post_b"]
    return z.transpose(0, 2, 1).reshape(B, D, BH, BW).astype(np.float32)


# revision 25
# speedup vs baseline: 2.7318x; 1.4342x over previous
"""CrossViewAttention Trainium2 kernel — full on-device pipeline.

8 NeuronCores = 2 batches x 4 ranks. Stage 1 (prep): each core BN+ReLU+conv's
its pixel slice of `feature` (420 px/camera), builds img/bev embeddings, and
LayerNorms key/val/query features — channels-on-partitions layout, with
cross-partition stats via ones-matmuls. Stage 2: one AllGather per batch
group shares the LN'd features. Stage 3: every core projects q/k/v with
host-sliced per-head weight inputs (so the SPMD graph stays static; the
core's head identity lives in its input data), then runs the attention
(scores, exp, P@[V|1] with fused softmax denominator) over all 6*1680 keys
for its (batch, head).

k_b provably cancels in the softmax (dropped); v_b commutes out of the
attention average (added on host); q_b is applied on device.

Host does only: tiny geometry einsums, input slicing/casts, and the output
proj+MLP (<1 GFLOP). The PJRT dispatch callable is built once and cached.

Self-contained: hardcodes all shapes; no sibling imports.
"""
import sys, os
sys.path.insert(0, "/opt/trn_rl_repo")

import numpy as np
import ml_dtypes
from scipy.special import erf

B, N, C_FEAT, FH, FW = 2, 6, 128, 28, 60
D, HEADS, DHEAD = 128, 4, 32
BH, BW = 32, 32
EPS = 1e-5
K = FH * FW            # 1680 keys per camera
Q = BH * BW            # 1024 queries
NK = N * K             # 10080
PCH = 105              # pixel chunk: 420 px/rank-cam = 4*105
NCH = NK // PCH        # 96 chunks
CH_PER_CAM = K // PCH  # 16
PXC = 420              # pixels per camera handled by one prep core
PPC = N * PXC          # 2520 prep pixels per core
QSL = Q // 4           # 256 query positions per prep core
QPC = N * QSL          # 1536 query rows per prep core
GW = 2 * PPC + QPC     # 6576 bounce columns (k | v | q)

_bf16 = ml_dtypes.bfloat16
_CACHE = {}


def _chunks(total, size):
    return [(s, min(size, total - s)) for s in range(0, total, size)]


def _build_nc():
    import concourse.tile as tile
    from concourse import bacc, mybir

    nc = bacc.Bacc("TRN2", target_bir_lowering=False, debug=False, num_devices=8)
    dt = mybir.dt
    AF = mybir.ActivationFunctionType

    def din(name, shape, d=dt.bfloat16):
        return nc.dram_tensor(name, shape, d, kind="ExternalInput").ap()

    feat = din("feat", [D, PPC])
    d_in = din("d_in", [4, PPC])
    x_sl = din("x_sl", [D, QSL])
    we_sl = din("we_sl", [D, QSL])
    cT = din("cT", [4, N])
    fp_wT = din("fp_wT", [D, D]); fl_wT = din("fl_wT", [D, D])
    img_wT = din("img_wT", [4, D]); cam_wT = din("cam_wT", [4, D])
    s_fp = din("s_fp", [D, 1], dt.float32); t_fp = din("t_fp", [D, 1], dt.float32)
    s_fl = din("s_fl", [D, 1], dt.float32); t_fl = din("t_fl", [D, 1], dt.float32)
    kg = din("kg", [D, 1], dt.float32); kb = din("kb", [D, 1], dt.float32)
    vg = din("vg", [D, 1], dt.float32); vb = din("vb", [D, 1], dt.float32)
    qg = din("qg", [D, 1], dt.float32); qb = din("qb", [D, 1], dt.float32)
    k_w_sl = din("k_w_sl", [D, DHEAD])
    v_w_sl = din("v_w_sl", [D, DHEAD])
    q_w_sl = din("q_w_sl", [D, DHEAD])
    q_b_sl = din("q_b_sl", [DHEAD, 1], dt.float32)
    av = nc.dram_tensor("av", [DHEAD + 1, Q], dt.float32, kind="ExternalOutput").ap()

    SCALE = 1.0 / np.sqrt(DHEAD)

    with tile.TileContext(nc) as tc:
        with (
            tc.tile_pool(name="persist", bufs=1) as pp,      # long-lived SBUF
            tc.tile_pool(name="wts", bufs=1) as wp,          # weights
            tc.tile_pool(name="dram", bufs=1, space="DRAM") as dramp,
        ):
            ones128 = wp.tile([D, 1], dt.bfloat16, tag="ones128")
            nc.vector.memset(ones128[:], 1.0)
            one1 = wp.tile([1, D], dt.bfloat16, tag="one1")
            nc.vector.memset(one1[:], 1.0)
            eps_ln = wp.tile([1, 1], dt.float32, tag="eps_ln")
            nc.vector.memset(eps_ln[:], EPS)
            eps7 = wp.tile([1, 1], dt.float32, tag="eps7")
            nc.vector.memset(eps7[:], 1e-7)

            def wload(ap, shape, d=dt.bfloat16):
                t = wp.tile(shape, d, tag=ap.tensor.name + "_sb")
                nc.sync.dma_start(t[:], ap[:])
                return t

            fp_wT_sb = wload(fp_wT, [D, D]); fl_wT_sb = wload(fl_wT, [D, D])
            img_wT_sb = wload(img_wT, [4, D]); cam_wT_sb = wload(cam_wT, [4, D])
            s_fp_sb = wload(s_fp, [D, 1], dt.float32); t_fp_sb = wload(t_fp, [D, 1], dt.float32)
            s_fl_sb = wload(s_fl, [D, 1], dt.float32); t_fl_sb = wload(t_fl, [D, 1], dt.float32)
            kg_sb = wload(kg, [D, 1], dt.float32); kb_sb = wload(kb, [D, 1], dt.float32)
            vg_sb = wload(vg, [D, 1], dt.float32); vb_sb = wload(vb, [D, 1], dt.float32)
            qg_sb = wload(qg, [D, 1], dt.float32); qb_sb = wload(qb, [D, 1], dt.float32)
            k_w_sb = wload(k_w_sl, [D, DHEAD])
            v_w_sb = wload(v_w_sl, [D, DHEAD])
            q_w_sb = wload(q_w_sl, [D, DHEAD])
            q_b_sb = wload(q_b_sl, [DHEAD, 1], dt.float32)

            # ---------------- stage 1: prep ----------------
            ln_k_bf = pp.tile([D, PPC], dt.bfloat16, tag="ln_k")
            ln_v_bf = pp.tile([D, PPC], dt.bfloat16, tag="ln_v")
            ln_q_bf = pp.tile([D, QPC], dt.bfloat16, tag="ln_q")

            with (
                tc.tile_pool(name="prep", bufs=1) as sp,       # scratch SBUF
                tc.tile_pool(name="pps", bufs=2, space="PSUM") as pps,
                tc.tile_pool(name="sps", bufs=2, space="PSUM") as sps,
            ):
                feat_sb = sp.tile([D, PPC], dt.bfloat16, tag="feat")
                nc.sync.dma_start(feat_sb[:], feat[:])
                d_sb = sp.tile([4, PPC], dt.bfloat16, tag="d")
                nc.sync.dma_start(d_sb[:], d_in[:])
                x_sb = sp.tile([D, QSL], dt.bfloat16, tag="x")
                nc.sync.dma_start(x_sb[:], x_sl[:])
                we_sb = sp.tile([D, QSL], dt.bfloat16, tag="we")
                nc.sync.dma_start(we_sb[:], we_sl[:])
                cT_sb = sp.tile([4, N], dt.bfloat16, tag="cT")
                nc.sync.dma_start(cT_sb[:], cT[:])
                def colnorm_inv(x_sb, M, eps_style):
                    """per-column 1/(||x||+1e-7) (eps_style='norm') or
                    rsqrt(mean(x^2)+EPS) (eps_style='ln') of centered input.
                    Returns bf16 [1, M] tile."""
                    sq = sp.tile([D, M], dt.bfloat16, tag="sq%d" % M)
                    nc.scalar.activation(sq[:], x_sb[:], AF.Square)
                    acc = sp.tile([1, M], dt.float32, tag="acc%d" % M)
                    for s, w in _chunks(M, 504):
                        ps = sps.tile([1, 504], dt.float32, tag="stat")
                        nc.tensor.matmul(ps[:, :w], ones128[:], sq[:, s:s + w],
                                         start=True, stop=True)
                        nc.vector.tensor_copy(acc[:, s:s + w], ps[:, :w])
                    inv = sp.tile([1, M], dt.bfloat16, tag="inv%d" % M)
                    nrm = sp.tile([1, M], dt.float32, tag="nrm%d" % M)
                    if eps_style == "ln":
                        nc.scalar.activation(nrm[:], acc[:], AF.Sqrt,
                                             scale=1.0 / D, bias=eps_ln[:])
                    else:
                        nc.scalar.activation(nrm[:], acc[:], AF.Sqrt)
                        nc.vector.tensor_scalar_add(nrm[:], nrm[:], eps7[:])
                    nc.vector.reciprocal(acc[:], nrm[:])
                    nc.vector.tensor_copy(inv[:], acc[:])
                    return inv

                def bcast_mul(dst_bf_or_f32, x_sb, inv_bf, M, g=None, b=None):
                    """dst = (x * bcast(inv)) [* g + b]; dst dtype per tile."""
                    for s, w in _chunks(M, 504):
                        ps = pps.tile([D, 504], dt.float32, tag="bc")
                        nc.tensor.matmul(ps[:, :w], one1[:], inv_bf[:, s:s + w],
                                         start=True, stop=True)
                        nc.vector.tensor_mul(ps[:, :w], x_sb[:, s:s + w], ps[:, :w])
                        if g is not None:
                            nc.vector.tensor_scalar(
                                dst_bf_or_f32[:, s:s + w], ps[:, :w],
                                g[:], b[:],
                                mybir.AluOpType.mult, mybir.AluOpType.add)
                        else:
                            nc.vector.tensor_copy(dst_bf_or_f32[:, s:s + w], ps[:, :w])

                def center(dst, x_sb, M):
                    """dst = x - colmean(x) (f32); x bf16-readable."""
                    xb = x_sb
                    if x_sb.tensor.dtype != dt.bfloat16:
                        xb = sp.tile([D, M], dt.bfloat16, tag="xb%d" % M)
                        nc.vector.tensor_copy(xb[:], x_sb[:])
                    mean = sp.tile([1, M], dt.bfloat16, tag="mean%d" % M)
                    for s, w in _chunks(M, 504):
                        ps = sps.tile([1, 504], dt.float32, tag="stat")
                        nc.tensor.matmul(ps[:, :w], ones128[:], xb[:, s:s + w],
                                         start=True, stop=True)
                        nc.vector.tensor_scalar_mul(mean[:, s:s + w], ps[:, :w],
                                                    1.0 / D)
                    for s, w in _chunks(M, 504):
                        ps = pps.tile([D, 504], dt.float32, tag="bc")
                        nc.tensor.matmul(ps[:, :w], one1[:], mean[:, s:s + w],
                                         start=True, stop=True)
                        nc.vector.tensor_sub(dst[:, s:s + w], x_sb[:, s:s + w],
                                             ps[:, :w])

                def layernorm(dst_bf, x_sb, M, g, b):
                    cen = sp.tile([D, M], dt.float32, tag="cen%d" % M)
                    center(cen, x_sb, M)
                    inv = colnorm_inv(cen, M, "ln")
                    bcast_mul(dst_bf, cen, inv, M, g, b)

                # img_embed: de = img_wT.T @ d ; -c_embed per cam; normalize
                c_emb = sp.tile([D, N], dt.float32, tag="c_emb")
                ps = pps.tile([D, 504], dt.float32, tag="bc")
                nc.tensor.matmul(ps[:, :N], cam_wT_sb[:], cT_sb[:],
                                 start=True, stop=True)
                nc.vector.tensor_copy(c_emb[:], ps[:, :N])

                img_e = sp.tile([D, PPC], dt.float32, tag="img_e")
                for s, w in _chunks(PPC, 504):
                    ps = pps.tile([D, 504], dt.float32, tag="bc")
                    nc.tensor.matmul(ps[:, :w], img_wT_sb[:], d_sb[:, s:s + w],
                                     start=True, stop=True)
                    nc.vector.tensor_copy(img_e[:, s:s + w], ps[:, :w])
                for cam in range(N):
                    nc.vector.tensor_scalar_sub(
                        img_e[:, cam * PXC:(cam + 1) * PXC],
                        img_e[:, cam * PXC:(cam + 1) * PXC],
                        c_emb[:, cam:cam + 1])
                inv = colnorm_inv(img_e, PPC, "norm")
                img_n = img_e
                bcast_mul(img_n, img_e, inv, PPC)

                # BN+ReLU (fused) then conv; fp-branch evac adds img_n
                kf = sp.tile([D, PPC], dt.bfloat16, tag="kf")
                vf = sp.tile([D, PPC], dt.bfloat16, tag="vf")
                relu = sp.tile([D, PPC], dt.bfloat16, tag="relu")
                for (s_ap, t_ap, w_sb, dst, add_img) in (
                    (s_fp_sb, t_fp_sb, fp_wT_sb, kf, True),
                    (s_fl_sb, t_fl_sb, fl_wT_sb, vf, False),
                ):
                    nc.scalar.activation(relu[:], feat_sb[:], AF.Relu,
                                         scale=s_ap[:], bias=t_ap[:])
                    for s, w in _chunks(PPC, 504):
                        ps = pps.tile([D, 504], dt.float32, tag="bc")
                        nc.tensor.matmul(ps[:, :w], w_sb[:], relu[:, s:s + w],
                                         start=True, stop=True)
                        if add_img:
                            nc.vector.tensor_add(dst[:, s:s + w], ps[:, :w],
                                                 img_n[:, s:s + w])
                        else:
                            nc.vector.tensor_copy(dst[:, s:s + w], ps[:, :w])

                layernorm(ln_k_bf, kf, PPC, kg_sb, kb_sb)
                layernorm(ln_v_bf, vf, PPC, vg_sb, vb_sb)

                # query: w_embed slice (host) - c_embed per cam, normalize, + x, LN
                qe = sp.tile([D, QPC], dt.float32, tag="qe")
                for cam in range(N):
                    nc.vector.tensor_scalar_sub(
                        qe[:, cam * QSL:(cam + 1) * QSL],
                        we_sb[:],
                        c_emb[:, cam:cam + 1])
                inv = colnorm_inv(qe, QPC, "norm")
                bcast_mul(qe, qe, inv, QPC)
                for cam in range(N):
                    nc.vector.tensor_add(qe[:, cam * QSL:(cam + 1) * QSL],
                                         qe[:, cam * QSL:(cam + 1) * QSL],
                                         x_sb[:])
                layernorm(ln_q_bf, qe, QPC, qg_sb, qb_sb)

            # ---------------- stage 2: collective ----------------
            ib = dramp.tile([D, GW], dt.bfloat16, tag="ib")
            ob = dramp.tile([4 * D, GW], dt.bfloat16, tag="ob")
            nc.gpsimd.dma_start(ib[:, 0:PPC], ln_k_bf[:])
            nc.gpsimd.dma_start(ib[:, PPC:2 * PPC], ln_v_bf[:])
            nc.gpsimd.dma_start(ib[:, 2 * PPC:GW], ln_q_bf[:])
            nc.gpsimd.collective_compute(
                "AllGather",
                mybir.AluOpType.bypass,
                replica_groups=[[0, 1, 2, 3], [4, 5, 6, 7]],
                ins=[ib.opt()],
                outs=[ob.opt()],
            )

            # ---------------- stage 3: assemble + project ----------------
            ap_pool = tc.tile_pool(name="att", bufs=1)
            ap = ap_pool.__enter__()
            kf_all = ap.tile([D, NK], dt.bfloat16, tag="kf_all")
            vf_all = ap.tile([D, NK], dt.bfloat16, tag="vf_all")
            qf_all = ap.tile([D, N * Q], dt.bfloat16, tag="qf_all")
            for r in range(4):
                rb = ob[r * D:(r + 1) * D, :]
                for cam in range(N):
                    nc.sync.dma_start(
                        kf_all[:, cam * K + r * PXC: cam * K + (r + 1) * PXC],
                        rb[:, cam * PXC:(cam + 1) * PXC])
                    nc.sync.dma_start(
                        vf_all[:, cam * K + r * PXC: cam * K + (r + 1) * PXC],
                        rb[:, PPC + cam * PXC: PPC + (cam + 1) * PXC])
                    nc.sync.dma_start(
                        qf_all[:, cam * Q + r * QSL: cam * Q + (r + 1) * QSL],
                        rb[:, 2 * PPC + cam * QSL: 2 * PPC + (cam + 1) * QSL])

            kh_sb = ap.tile([DHEAD, NK], dt.bfloat16, tag="kh")
            qh_sb = ap.tile([DHEAD, N * Q], dt.bfloat16, tag="qh")
            vo_sb = ap.tile([PCH, NCH * (DHEAD + 1)], dt.bfloat16, tag="vo")

            with tc.tile_pool(name="prjps", bufs=3, space="PSUM") as prps:
                for s, w in _chunks(NK, 504):
                    ps = prps.tile([DHEAD, 512], dt.float32, tag="prj")
                    nc.tensor.matmul(ps[:, :w], k_w_sb[:], kf_all[:, s:s + w],
                                     start=True, stop=True)
                    nc.vector.tensor_copy(kh_sb[:, s:s + w], ps[:, :w])
                for s, w in _chunks(N * Q, 512):
                    ps = prps.tile([DHEAD, 512], dt.float32, tag="prj")
                    nc.tensor.matmul(ps[:, :w], q_w_sb[:], qf_all[:, s:s + w],
                                     start=True, stop=True)
                    nc.vector.tensor_scalar_add(qh_sb[:, s:s + w], ps[:, :w],
                                                q_b_sb[:])
                nc.vector.memset(vo_sb[:, DHEAD::DHEAD + 1], 1.0)
                for c in range(NCH):
                    ps = prps.tile([PCH, DHEAD], dt.float32, tag="vprj")
                    nc.tensor.matmul(ps[:], vf_all[:, c * PCH:(c + 1) * PCH],
                                     v_w_sb[:], start=True, stop=True)
                    nc.vector.tensor_copy(
                        vo_sb[:, c * (DHEAD + 1): c * (DHEAD + 1) + DHEAD], ps[:])

            # ---------------- stage 4: attention ----------------
            with (
                tc.tile_pool(name="p", bufs=3) as p_pool,
                tc.tile_pool(name="ps", bufs=2, space="PSUM") as ps_pool,
                tc.tile_pool(name="avp", bufs=1, space="PSUM") as av_pool,
                tc.tile_pool(name="outs", bufs=1) as out_pool,
            ):
                av_ps = av_pool.tile([DHEAD + 1, Q], dt.float32)
                for c in range(NCH):
                    cam = c // CH_PER_CAM
                    s_ps = ps_pool.tile([PCH, Q], dt.float32, tag="scores")
                    for half in range(2):
                        nc.tensor.matmul(
                            s_ps[:, half * 512:(half + 1) * 512],
                            kh_sb[:, c * PCH:(c + 1) * PCH],
                            qh_sb[:, cam * Q + half * 512: cam * Q + (half + 1) * 512],
                            start=True, stop=True,
                        )
                    p_sb = p_pool.tile([PCH, Q], dt.bfloat16, tag="p")
                    nc.scalar.activation(p_sb[:], s_ps[:], AF.Exp, scale=SCALE)
                    for half in range(2):
                        nc.tensor.matmul(
                            av_ps[:, half * 512:(half + 1) * 512],
                            vo_sb[:, c * (DHEAD + 1):(c + 1) * (DHEAD + 1)],
                            p_sb[:, half * 512:(half + 1) * 512],
                            start=(c == 0), stop=(c == NCH - 1),
                        )
                av_sb = out_pool.tile([DHEAD + 1, Q], dt.float32)
                nc.vector.tensor_copy(av_sb[:], av_ps[:])
                nc.sync.dma_start(av[:], av_sb[:])
            ap_pool.__exit__(None, None, None)

    nc.compile()
    return nc


def _build_dispatch(nc):
    """Build the sharded PJRT callable ONCE (what run_bass_kernel_spmd
    re-creates per call under axon) and return a fast-path runner."""
    import jax
    from jax.sharding import Mesh, PartitionSpec
    from jax.experimental.shard_map import shard_map
    from concourse.bass2jax import (
        _bass_exec_p, install_neuronx_cc_hook, partition_id_tensor,
    )
    from concourse import mybir

    install_neuronx_cc_hook()
    partition_name = nc.partition_id_tensor.name if nc.partition_id_tensor else None
    in_names, out_names, out_avals, zero_shapes = [], [], [], []
    for alloc in nc.m.functions[0].allocations:
        if not isinstance(alloc, mybir.MemoryLocationSet):
            continue
        name = alloc.memorylocations[0].name
        if alloc.kind == "ExternalInput":
            if name != partition_name:
                in_names.append(name)
        elif alloc.kind == "ExternalOutput":
            shape = tuple(alloc.tensor_shape)
            dtype = mybir.dt.np(alloc.dtype)
            out_names.append(name)
            out_avals.append(jax.core.ShapedArray(shape, dtype))
            zero_shapes.append((shape, dtype))
    n_params = len(in_names)
    n_outs = len(out_avals)
    in_names_all = in_names + out_names
    if partition_name is not None:
        in_names_all.append(partition_name)
    donate = tuple(range(n_params, n_params + n_outs))

    def _body(*args):
        operands = list(args)
        if partition_name is not None:
            operands.append(partition_id_tensor())
        outs = _bass_exec_p.bind(
            *operands,
            out_avals=tuple(out_avals),
            in_names=tuple(in_names_all),
            out_names=tuple(out_names),
            lowering_input_output_aliases=(),
            sim_require_finite=True,
            sim_require_nnan=True,
            nc=nc,
        )
        return tuple(outs)

    devices = jax.devices()[:8]
    mesh = Mesh(np.asarray(devices), ("core",))
    in_specs = (PartitionSpec("core"),) * (n_params + n_outs)
    out_specs = (PartitionSpec("core"),) * len(out_names)
    sharded = jax.jit(
        shard_map(_body, mesh=mesh, in_specs=in_specs, out_specs=out_specs,
                  check_rep=False),
        donate_argnums=donate,
        keep_unused=True,
    )

    def run(in_maps):
        concat_in = [
            np.concatenate([np.asarray(m[name]) for m in in_maps], axis=0)
            for name in in_names
        ]
        concat_zeros = [
            np.zeros((8 * s[0], *s[1:]), dt) for s, dt in zero_shapes
        ]
        out_arrs = sharded(*concat_in, *concat_zeros)
        return [
            {
                name: np.asarray(out_arrs[i]).reshape(8, *out_avals[i].shape)[c]
                for i, name in enumerate(out_names)
            }
            for c in range(8)
        ]

    return run


def kernel(**inputs):
    inp = {k: np.asarray(v, dtype=np.float32) for k, v in inputs.items()}
    x = inp["x"]; feature = inp["feature"]; I_inv = inp["I_inv"]; E_inv = inp["E_inv"]
    bev_grid = inp["bev_grid"]; image_plane = inp["image_plane"]

    # --- host: tiny geometry (rays only) ---
    pixp = image_plane.reshape(3, K)
    cam = np.einsum("bnij,jk->bnik", I_inv, pixp)
    cam = np.concatenate([cam, np.ones((B, N, 1, K), np.float32)], 2)
    d = np.einsum("bnij,bnjk->bnik", E_inv, cam)              # (b,n,4,K)

    # --- host: per-core in_maps ---
    def fold_bn(p):
        s = inp[p + "_bn_g"] / np.sqrt(inp[p + "_bn_v"] + EPS)
        t = inp[p + "_bn_b"] - inp[p + "_bn_m"] * s
        return s.reshape(D, 1), t.reshape(D, 1)
    s_fp, t_fp = fold_bn("fp"); s_fl, t_fl = fold_bn("fl")

    w_embed = (inp["bev_w"] @ bev_grid[:2].reshape(2, Q)
               + inp["bev_b"][:, None]).astype(_bf16)          # [128, 1024]
    common = {
        "fp_wT": inp["fp_w"].T.astype(_bf16),
        "fl_wT": inp["fl_w"].T.astype(_bf16),
        "img_wT": inp["img_w"].T.astype(_bf16),
        "cam_wT": inp["cam_w"].T.astype(_bf16),
        "s_fp": s_fp, "t_fp": t_fp, "s_fl": s_fl, "t_fl": t_fl,
        "kg": inp["k_ln_g"].reshape(D, 1), "kb": inp["k_ln_b"].reshape(D, 1),
        "vg": inp["v_ln_g"].reshape(D, 1), "vb": inp["v_ln_b"].reshape(D, 1),
        "qg": inp["q_ln_g"].reshape(D, 1), "qb": inp["q_ln_b"].reshape(D, 1),
    }
    fT = feature.reshape(B, N, D, K)
    dT = d
    in_maps = []
    for core in range(8):
        b, j = core // 4, core % 4
        m = dict(common)
        m["feat"] = np.ascontiguousarray(
            fT[b, :, :, j * PXC:(j + 1) * PXC].transpose(1, 0, 2).reshape(D, PPC)
        ).astype(_bf16)
        m["d_in"] = np.ascontiguousarray(
            dT[b, :, :, j * PXC:(j + 1) * PXC].transpose(1, 0, 2).reshape(4, PPC)
        ).astype(_bf16)
        m["x_sl"] = np.ascontiguousarray(
            x[b].reshape(D, Q)[:, j * QSL:(j + 1) * QSL]).astype(_bf16)
        m["we_sl"] = np.ascontiguousarray(w_embed[:, j * QSL:(j + 1) * QSL])
        m["cT"] = np.ascontiguousarray(E_inv[b, :, :, -1].T).astype(_bf16)
        h = core % HEADS
        sl = slice(h * DHEAD, (h + 1) * DHEAD)
        m["k_w_sl"] = np.ascontiguousarray(inp["k_w"][:, sl]).astype(_bf16)
        m["v_w_sl"] = np.ascontiguousarray(inp["v_w"][:, sl]).astype(_bf16)
        m["q_w_sl"] = np.ascontiguousarray(inp["q_w"][:, sl]).astype(_bf16)
        m["q_b_sl"] = np.ascontiguousarray(inp["q_b"][sl].reshape(DHEAD, 1))
        in_maps.append(m)

    if os.environ.get("KERNEL_EMULATE"):
        avs = _emulate(in_maps)
    else:
        import time
        if "run" not in _CACHE:
            _CACHE["nc"] = _build_nc()
            _CACHE["run"] = _build_dispatch(_CACHE["nc"])
        t0 = time.time()
        res = _CACHE["run"](in_maps)
        _CACHE["device_wall_s"] = time.time() - t0
        avs = [res[i]["av"] for i in range(8)]

    # --- host: combine heads (+v_b), proj, MLP ---
    a = np.empty((B, Q, HEADS, DHEAD), np.float32)
    for core in range(8):
        b, h = core // HEADS, core % HEADS
        av = avs[core]
        a[b, :, h, :] = (av[:DHEAD] / av[DHEAD:DHEAD + 1]).T \
            + inp["v_b"][h * DHEAD:(h + 1) * DHEAD]
    a = a.reshape(B, Q, HEADS * DHEAD)
    z = a @ inp["proj_w"] + inp["proj_b"]
    z += x.reshape(B, D, Q).transpose(0, 2, 1)
    mu = z.mean(-1, keepdims=True)
    z -= mu
    var = np.einsum("bqc,bqc->bq", z, z) / D
    z *= (1.0 / np.sqrt(var + EPS))[..., None]
    z = z * inp["pre_g"] + inp["pre_b"]
    h1 = z @ inp["mlp_w1"] + inp["mlp_b1"]
    h1 = 0.5 * h1 * (1.0 + erf(h1 * np.float32(1.0 / np.sqrt(2.0))))
    z = z + h1 @ inp["mlp_w2"] + inp["mlp_b2"]
    mu = z.mean(-1, keepdims=True)
    z -= mu
    var = np.einsum("bqc,bqc->bq", z, z) / D
    z *= (1.0 / np.sqrt(var + EPS))[..., None]
    z = z * inp["post_g"] + inp["post_b"]
    return z.transpose(0, 2, 1).reshape(B, D, BH, BW).astype(np.float32)


def _emulate(in_maps):
    """Numpy mirror of the device graph (layouts included) for debugging."""
    f32 = np.float32
    def bf(a):
        return a.astype(_bf16).astype(f32)

    bounces = []
    for core in range(8):
        m = {k: v.astype(f32) for k, v in in_maps[core].items()}
        # prep
        c_emb = m["cam_wT"].T @ m["cT"]                        # [128, 6]
        img_e = m["img_wT"].T @ m["d_in"]                      # [128, 2520]
        for camn in range(N):
            img_e[:, camn * PXC:(camn + 1) * PXC] -= c_emb[:, camn:camn + 1]
        img_n = img_e / (np.sqrt((img_e ** 2).sum(0, keepdims=True)) + 1e-7)
        def branch(p):
            th = np.maximum(m["feat"] * m["s_" + p] + m["t_" + p], 0.0)
            return m[p + "_wT"].T @ bf(th)
        kf = branch("fp") + img_n
        vf = branch("fl")
        def ln(xx, g, b):
            mu = xx.mean(0, keepdims=True)
            cen = xx - mu
            rstd = 1.0 / np.sqrt((cen ** 2).mean(0, keepdims=True) + EPS)
            return bf(cen * rstd * g + b)
        ln_k = ln(kf, m["kg"], m["kb"])
        ln_v = ln(vf, m["vg"], m["vb"])
        qe = np.concatenate(
            [m["we_sl"] - c_emb[:, camn:camn + 1] for camn in range(N)], 1)
        qn = qe / (np.sqrt((qe ** 2).sum(0, keepdims=True)) + 1e-7)
        qn = qn + np.tile(m["x_sl"], (1, N))
        ln_q = ln(qn, m["qg"], m["qb"])
        bounces.append(np.concatenate([ln_k, ln_v, ln_q], 1))  # [128, GW]

    avs = []
    for core in range(8):
        b = core // 4
        m = {k: v.astype(f32) for k, v in in_maps[core].items()}
        gathered = [bounces[4 * b + r] for r in range(4)]
        kf_all = np.empty((D, NK), f32)
        vf_all = np.empty((D, NK), f32)
        qf_all = np.empty((D, N * Q), f32)
        for r in range(4):
            rb = gathered[r]
            for camn in range(N):
                kf_all[:, camn * K + r * PXC: camn * K + (r + 1) * PXC] = \
                    rb[:, camn * PXC:(camn + 1) * PXC]
                vf_all[:, camn * K + r * PXC: camn * K + (r + 1) * PXC] = \
                    rb[:, PPC + camn * PXC: PPC + (camn + 1) * PXC]
                qf_all[:, camn * Q + r * QSL: camn * Q + (r + 1) * QSL] = \
                    rb[:, 2 * PPC + camn * QSL: 2 * PPC + (camn + 1) * QSL]
        kh = bf(m["k_w_sl"].T @ kf_all)                        # [32, NK]
        qh = bf(m["q_w_sl"].T @ qf_all + m["q_b_sl"])          # [32, 6144]
        vo = bf(vf_all.T @ m["v_w_sl"])                        # [NK, 32]
        av = np.zeros((DHEAD + 1, Q), f32)
        for camn in range(N):
            sc = kh[:, camn * K:(camn + 1) * K].T @ qh[:, camn * Q:(camn + 1) * Q]
            p = np.exp(sc / np.sqrt(DHEAD))
            av[:DHEAD] += vo[camn * K:(camn + 1) * K].T @ p
            av[DHEAD] += p.sum(0)
        avs.append(av)
    return avs
